# revision 7
# baseline (speedup 1.0000x reference)
"""Trainium2 Bass kernel for BoundaryLoss (data-parallel over batch).

Math (per batch sample b):
  mask  = boundary mask of target = (maxpool5x5(t) != minpool5x5(t)) with
          cv2-style clipped windows (OOB ignored).  Equals the reference's
          per-class dilate/erode union because a 5x5 window is non-uniform
          iff some class boundary passes through it.
  ce    = logsumexp_c(pred) - pred[t]
  wsum  = sum(mask * ce);  msum = sum(mask)
  per_sample = msum > 0 ? wsum/max(msum,1) : wsum/(H*W);  out = mean_b

Device algorithm (one sample per core), v2:
  - pred streams in via SWDGE *casting* DMA (fp32 HBM -> fp16 SBUF) in
    "layout B" [128, (4 rows, 512)] (partition p = rows 4p..4p+3), 8KB
    contiguous read runs.  The 21 MB HBM read is the roofline.  All
    on-chip tensors are 16-bit so DVE ops hit the 2x/4x perf modes.
  - S = sum_c exp(pred_c): exp on ACT emitting fp8e4 directly; summed
    over class PAIRS by DoubleRow identity-matmuls (2 k-tiles per pass,
    0.5 cyc/row) accumulating in PSUM -- 4x less PE time than per-class
    fp16 identity matmuls.
  - picked = pred[t] accumulated per-pixel in PSUM ("P"): per class one
    fused scalar_tensor_tensor (tb==c)*p16_c on DVE (4x mode, all fp16
    SBUF) + 4 fp16 identity matmuls.  No exp/ln round-trip on this path.
  - boundary mask entirely in layout B (no PE transposes, no DRAM
    bounce, single target load): horizontal 5-pools via shifted
    tensor_tensor on padded rows; vertical 5-pools on an 8-row extended
    tile whose 2 boundary rows come from partition-shifted SBUF->SBUF
    DMAs (sync queue, off the SWDGE pred stream).
  - finals: ln(S) on ACT (fp16 out), stt accums for sum(mask*lse),
    sum(mask*picked), sum(mask); partition-reduce via ones-matmuls;
    DMA out [1,32].  Host combines per-core outputs.
"""

import numpy as np

B = 8
C = 21
H = 512
W = 512
N_CORES = 8
G4 = 4          # row groups (H = 128 * G4)
PW = 520        # padded width for horizontal pooling; data cols [2, 514)
CHUNKS = [2, 2, 4, 4, 4, 4, 1]  # class chunking (sum = C)

_CACHE = {}


def _patch_act_tables(bacc_mod, mybir, arch):
    """Steer the act-table-load pass to the set containing BOTH exp and ln
    (one table load total instead of an exp-table load up front plus an
    ln-table reload in the serial tail): present every other set as empty
    so the greedy selection can only pick the combined one."""
    try:
        from concourse.hw_specs import get_activation_tables

        orig = get_activation_tables(arch)
        Fn = mybir.ActivationFunctionType
        need = {Fn.Exp, Fn.Ln, Fn.Copy}
        combined = next(name for name, fns in orig.items()
                        if need.issubset(fns))
        tables = {name: (fns if name == combined else set())
                  for name, fns in orig.items()}
        bacc_mod.get_activation_tables = lambda _arch: tables
    except Exception:
        pass


def _build_nc():
    from contextlib import ExitStack

    import concourse.bacc as bacc
    import concourse.tile as tile
    from concourse import mybir
    from concourse.masks import make_identity

    dt = mybir.dt
    Alu = mybir.AluOpType
    Act = mybir.ActivationFunctionType

    nc = bacc.Bacc("TRN2", target_bir_lowering=False, debug=False,
                   num_devices=N_CORES)
    _patch_act_tables(bacc, mybir, nc.m.arch)

    pred = nc.dram_tensor("pred", [C, H, W], dt.float32, kind="ExternalInput")
    target = nc.dram_tensor("target", [H, W], dt.int32, kind="ExternalInput")
    out = nc.dram_tensor("out", [1, 32], dt.float32, kind="ExternalOutput")

    with tile.TileContext(nc) as tc, ExitStack() as ctx:
        consts = ctx.enter_context(tc.tile_pool(name="consts", bufs=1))
        keep = ctx.enter_context(tc.tile_pool(name="keep", bufs=1))
        mp = ctx.enter_context(tc.tile_pool(name="maskpool", bufs=1))
        ms = ctx.enter_context(tc.tile_pool(name="maskscratch", bufs=1))
        epool = ctx.enter_context(tc.tile_pool(name="ep", bufs=2))
        opool = ctx.enter_context(tc.tile_pool(name="op", bufs=4))
        fin = ctx.enter_context(tc.tile_pool(name="fin", bufs=1))
        sps = ctx.enter_context(tc.tile_pool(name="spsum", bufs=1,
                                             space="PSUM"))
        pps = ctx.enter_context(tc.tile_pool(name="ppsum", bufs=1,
                                             space="PSUM"))

        ident16 = consts.tile([128, 128], dt.float16)
        make_identity(nc, ident16)
        ident8x2 = consts.tile([128, 2, 128], dt.float8e4)
        make_identity(nc, ident8x2[:, 0, :])
        make_identity(nc, ident8x2[:, 1, :])
        ones = consts.tile([128, 1], dt.float32)
        nc.gpsimd.memset(ones, 1.0)
        warm = consts.tile([128, 512], dt.float16)
        nc.gpsimd.memset(warm, 0.0)
        st_w1 = consts.tile([128, 1], dt.float32)   # sum mask*lse
        st_l2 = consts.tile([128, 1], dt.float32)   # sum mask*picked
        st_m = consts.tile([128, 1], dt.float32)    # sum mask

        # ---------------- resident tensors (layout B) ----------------
        p16 = keep.tile([128, C, G4, W], dt.float16)   # pred, fp16
        tb = keep.tile([128, G4, W], dt.float16)       # target as fp16
        maskb = keep.tile([128, G4, W], dt.float16)    # boundary mask

        # ---------------- early loads ----------------
        t32b = mp.tile([128, G4, W], dt.int32, tag="t32b")
        nc.sync.dma_start(
            out=t32b, in_=target.ap().rearrange("(p r) w -> p r w", p=128))

        # pred chunk DMAs: issue ALL up front on the SWDGE queue so the
        # descriptor stream never bubbles (p16 is resident, no pool deps).
        starts = []
        c0 = 0
        for n in CHUNKS:
            starts.append((c0, n))
            nc.gpsimd.dma_start(
                out=p16[:, c0:c0 + n, :, :],
                in_=pred.ap()[c0:c0 + n].rearrange(
                    "c (p r) w -> p c r w", p=128))
            c0 += n

        nc.vector.tensor_copy(out=tb, in_=t32b)

        # PE warmup into the future P bank (discarded by c==0's start=True)
        s_ps = sps.tile([128, G4, W], dt.float32, tag="s")
        p_ps = pps.tile([128, G4, W], dt.float32, tag="p")
        for _ in range(10):
            nc.tensor.matmul(p_ps[:, 0, :], ident16, warm, start=True,
                             stop=True)

        # ---------------- mask pipeline (all layout B) ----------------
        xmax = mp.tile([128, G4, PW], dt.float16, tag="xmax")
        xmin = mp.tile([128, G4, PW], dt.float16, tag="xmin")
        for t, v in ((xmax, -1.0), (xmin, 99.0)):
            nc.gpsimd.memset(t[:, :, 0:2], v)
            nc.gpsimd.memset(t[:, :, 2 + W:PW], v)
        # extended row tiles: rows 0..7 = global rows 4p-2 .. 4p+5
        extx = mp.tile([128, 8, W], dt.float16, tag="extx")
        extn = mp.tile([128, 8, W], dt.float16, tag="extn")
        # border rows for boundary partitions.  Engine ops must start at a
        # 32-aligned partition, so memset the whole last block; the later
        # boundary DMA overwrites partitions 96..126 with real data and
        # leaves partition 127 at the border value.
        nc.gpsimd.memset(extx[0:1, 0:2, :], -1.0)
        nc.gpsimd.memset(extx[96:128, 6:8, :], -1.0)
        nc.gpsimd.memset(extn[0:1, 0:2, :], 99.0)
        nc.gpsimd.memset(extn[96:128, 6:8, :], 99.0)
        vx = mp.tile([128, G4, W], dt.float16, tag="vx")
        vn = mp.tile([128, G4, W], dt.float16, tag="vn")
        junk = mp.tile([128, G4, W], dt.float16, tag="junk")

        def st_fill():
            # pad buffers for horizontal pooling (gpsimd: off DVE/ACT)
            nc.gpsimd.tensor_copy(out=xmax[:, :, 2:2 + W], in_=tb)
            nc.gpsimd.tensor_copy(out=xmin[:, :, 2:2 + W], in_=tb)

        def hpool(src, op, dst):
            # horizontal 5-pool into ext rows 2..6 (global rows 4p..4p+3)
            m2 = ms.tile([128, G4, PW], dt.float16, tag="m2")
            m4 = ms.tile([128, G4, PW], dt.float16, tag="m4")
            nc.vector.tensor_tensor(
                out=m2[:, :, 0:PW - 1],
                in0=src[:, :, 0:PW - 1], in1=src[:, :, 1:PW], op=op)
            nc.vector.tensor_tensor(
                out=m4[:, :, 0:PW - 3],
                in0=m2[:, :, 0:PW - 3], in1=m2[:, :, 2:PW - 1], op=op)
            nc.vector.tensor_tensor(
                out=dst, in0=m4[:, :, 0:W], in1=src[:, :, 4:4 + W], op=op)

        def st_bdry(ext):
            # boundary rows from neighbor partitions via SBUF->SBUF DMA
            # (sync queue; not the SWDGE pred stream).
            # ext[p, 0:2] = rows 4p-2,4p-1 = ext[p-1, 4:6]
            nc.sync.dma_start(out=ext[1:128, 0:2, :], in_=ext[0:127, 4:6, :])
            # ext[p, 6:8] = rows 4p+4,4p+5 = ext[p+1, 2:4]
            nc.sync.dma_start(out=ext[0:127, 6:8, :], in_=ext[1:128, 2:4, :])

        def vpool(ext, op, dst):
            # vertical 5-pool: out row r needs ext rows r..r+4
            m2 = ms.tile([128, 7, W], dt.float16, tag="v2")
            m4 = ms.tile([128, 5, W], dt.float16, tag="v4")
            nc.vector.tensor_tensor(
                out=m2, in0=ext[:, 0:7, :], in1=ext[:, 1:8, :], op=op)
            nc.vector.tensor_tensor(
                out=m4, in0=m2[:, 0:5, :], in1=m2[:, 2:7, :], op=op)
            nc.vector.tensor_tensor(
                out=dst, in0=m4[:, 0:4, :], in1=ext[:, 4:8, :], op=op)

        def st_neq():
            nc.vector.tensor_tensor(out=maskb, in0=vx, in1=vn,
                                    op=Alu.not_equal)

        def st_msum():
            nc.vector.tensor_scalar(
                out=junk, in0=maskb, scalar1=1.0, scalar2=0.0,
                op0=Alu.mult, op1=Alu.add, accum_out=st_m)

        stages = [
            st_fill,
            lambda: hpool(xmax, Alu.max, extx[:, 2:6, :]),
            lambda: hpool(xmin, Alu.min, extn[:, 2:6, :]),
            lambda: st_bdry(extx),
            lambda: st_bdry(extn),
            lambda: vpool(extx, Alu.max, vx),
            lambda: vpool(extn, Alu.min, vn),
            st_neq,
            st_msum,
        ]

        # ---------------- class loop, stages interleaved ----------------
        DR = mybir.MatmulPerfMode.DoubleRow
        for k, (c0, nct) in enumerate(starts):
            if k < len(stages):
                stages[k]()
            e8 = epool.tile([128, 4, G4, W], dt.float8e4, tag="e")
            nc.scalar.activation(out=e8[:, 0:nct, :, :],
                                 in_=p16[:, c0:c0 + nct, :, :], func=Act.Exp)
            # S accumulation: DoubleRow over class pairs (fp8, 2 k-tiles)
            for a in range(0, nct - 1, 2):
                for j in range(G4):
                    nc.tensor.matmul(
                        s_ps[:, j, :], ident8x2, e8[:, a:a + 2, j, :],
                        start=(c0 + a == 0), stop=False, perf_mode=DR)
            if nct % 2:  # solo class (the last chunk)
                for j in range(G4):
                    nc.tensor.matmul(
                        s_ps[:, j, :], ident8x2[:, 0, :],
                        e8[:, nct - 1, j, :],
                        start=(c0 + nct - 1 == 0), stop=(c0 + nct == C))
            # picked accumulation: fused (tb==c)*p16 on DVE + fp16 matmuls
            for i in range(nct):
                c = c0 + i
                o_t = opool.tile([128, G4, W], dt.float16, tag="o")
                nc.vector.scalar_tensor_tensor(
                    out=o_t, in0=tb, scalar=float(c),
                    in1=p16[:, c, :, :], op0=Alu.is_equal, op1=Alu.mult)
                for j in range(G4):
                    nc.tensor.matmul(
                        p_ps[:, j, :], ident16, o_t[:, j, :],
                        start=(c == 0), stop=(c == C - 1))
        for k in range(len(starts), len(stages)):
            stages[k]()

        # ---------------- finals ----------------
        l1 = fin.tile([128, G4, W], dt.float16)
        nc.scalar.activation(out=l1, in_=s_ps, func=Act.Ln)
        nc.vector.scalar_tensor_tensor(
            out=junk, in0=l1, scalar=0.0, in1=maskb,
            op0=Alu.add, op1=Alu.mult, accum_out=st_w1)
        nc.vector.scalar_tensor_tensor(
            out=junk, in0=p_ps, scalar=0.0, in1=maskb,
            op0=Alu.add, op1=Alu.mult, accum_out=st_l2)

        # partition reductions — reuse the S bank (fully consumed by l1)
        red = s_ps[0:1, 0, 0:32]
        nc.tensor.matmul(red[:, 0:1], ones, st_w1, start=True, stop=True)
        nc.tensor.matmul(red[:, 1:2], ones, st_l2, start=True, stop=True)
        nc.tensor.matmul(red[:, 2:3], ones, st_m, start=True, stop=True)
        outsb = consts.tile([1, 32], dt.float32)
        nc.vector.memset(outsb, 0.0)
        nc.vector.tensor_copy(out=outsb[:, 0:3], in_=red[:, 0:3])
        nc.sync.dma_start(out=out.ap(), in_=outsb)

    nc.compile()
    return nc


def get_nc():
    if "nc" not in _CACHE:
        _CACHE["nc"] = _build_nc()
    return _CACHE["nc"]


def _combine(outs):
    """outs: list of per-core [1,32] float32 -> scalar loss."""
    per_sample = []
    for o in outs:
        w1, l2, msum = float(o[0, 0]), float(o[0, 1]), float(o[0, 2])
        wsum = w1 - l2
        if msum > 0:
            per_sample.append(wsum / max(msum, 1.0))
        else:
            per_sample.append(wsum / float(H * W))
    return np.float32(np.mean(per_sample))


def kernel(pred, target):
    from concourse.bass_utils import run_bass_kernel_spmd

    pred = np.ascontiguousarray(pred, dtype=np.float32)
    target = np.ascontiguousarray(target, dtype=np.int32)
    assert pred.shape == (B, C, H, W) and target.shape == (B, H, W)

    nc = get_nc()
    in_maps = [{"pred": pred[b], "target": target[b]} for b in range(B)]
    res = run_bass_kernel_spmd(nc, in_maps, core_ids=list(range(N_CORES)))
    outs = [res.results[b]["out"] for b in range(B)]
    return np.asarray(_combine(outs), dtype=np.float32)


# revision 9
# speedup vs baseline: 1.0067x; 1.0067x over previous
"""Trainium2 Bass kernel for BoundaryLoss (data-parallel over batch).

Math (per batch sample b):
  mask  = boundary mask of target = (maxpool5x5(t) != minpool5x5(t)) with
          cv2-style clipped windows (OOB ignored).  Equals the reference's
          per-class dilate/erode union because a 5x5 window is non-uniform
          iff some class boundary passes through it.
  ce    = logsumexp_c(pred) - pred[t]
  wsum  = sum(mask * ce);  msum = sum(mask)
  per_sample = msum > 0 ? wsum/max(msum,1) : wsum/(H*W);  out = mean_b

Device algorithm (one sample per core), v3 (measured-rate tuned):
  - pred streams in via SWDGE *casting* DMA (fp32 HBM -> fp16 SBUF) in
    "layout B" [128, (4 rows, 512)] (partition p = rows 4p..4p+3), 8KB
    contiguous read runs, resident p16.  The 21 MB HBM read is the
    roofline.
  - S = sum_c exp(pred_c): exp on ACT emitting fp8e4 directly; summed
    over class PAIRS by DoubleRow identity-matmuls (2 k-tiles per pass,
    0.5 cyc/row) accumulating in PSUM.
  - picked = pred[t] per-pixel in PSUM ("P"): per class eq =
    tensor_scalar is_equal (DVE 4x, 0.69us) then o = eq*p16 tensor_tensor
    (DVE 2x, 1.18us) + 4 fp16 identity matmuls.  (stt / accum / select /
    copy_predicated all measured 1x on HW -- avoid them in the loop.)
  - boundary mask entirely in layout B, scheduled EARLY: two padded
    [128, 8, 520] tiles (rows 0..7 = global 4p-2..4p+5) built from two
    int32->fp16 casts + border memsets + partition-shift SBUF->SBUF DMAs
    on the sync queue right after the target cast (no mid-loop deps);
    horizontal 5-pools (8 rows), vertical 5-pools via strided row trees,
    neq -> maskb.  msum on ACT (activation accum).  No PSUM, no PE, no
    DRAM bounce for the mask.
  - finals: ln(S) on ACT (fp16 out), j2 = sum(mask*P) emitted before
    j1 = sum(mask*lse) so j2 overlaps the Ln; partition-reduce via
    ones-matmuls; DMA out [1,32].  Host combines per-core outputs.
"""

import numpy as np

B = 8
C = 21
H = 512
W = 512
N_CORES = 8
G4 = 4          # row groups (H = 128 * G4)
PW = 520        # padded width for horizontal pooling; data cols [2, 514)
CHUNKS = [2, 2, 4, 4, 4, 4, 1]  # class chunking (sum = C)

_CACHE = {}


def _patch_act_tables(bacc_mod, mybir, arch):
    """Steer the act-table-load pass to the set containing BOTH exp and ln
    (one table load total instead of an exp-table load up front plus an
    ln-table reload in the serial tail): present every other set as empty
    so the greedy selection can only pick the combined one."""
    try:
        from concourse.hw_specs import get_activation_tables

        orig = get_activation_tables(arch)
        Fn = mybir.ActivationFunctionType
        need = {Fn.Exp, Fn.Ln, Fn.Copy}
        combined = next(name for name, fns in orig.items()
                        if need.issubset(fns))
        tables = {name: (fns if name == combined else set())
                  for name, fns in orig.items()}
        bacc_mod.get_activation_tables = lambda _arch: tables
    except Exception:
        pass


def _build_nc():
    from contextlib import ExitStack

    import concourse.bacc as bacc
    import concourse.tile as tile
    from concourse import mybir
    from concourse.masks import make_identity

    dt = mybir.dt
    Alu = mybir.AluOpType
    Act = mybir.ActivationFunctionType

    nc = bacc.Bacc("TRN2", target_bir_lowering=False, debug=False,
                   num_devices=N_CORES)
    _patch_act_tables(bacc, mybir, nc.m.arch)

    pred = nc.dram_tensor("pred", [C, H, W], dt.float32, kind="ExternalInput")
    target = nc.dram_tensor("target", [H, W], dt.int32, kind="ExternalInput")
    out = nc.dram_tensor("out", [1, 32], dt.float32, kind="ExternalOutput")

    with tile.TileContext(nc) as tc, ExitStack() as ctx:
        consts = ctx.enter_context(tc.tile_pool(name="consts", bufs=1))
        keep = ctx.enter_context(tc.tile_pool(name="keep", bufs=1))
        mp = ctx.enter_context(tc.tile_pool(name="maskpool", bufs=1))
        ms = ctx.enter_context(tc.tile_pool(name="maskscratch", bufs=1))
        epool = ctx.enter_context(tc.tile_pool(name="ep", bufs=2))
        opool = ctx.enter_context(tc.tile_pool(name="op", bufs=2))
        fin = ctx.enter_context(tc.tile_pool(name="fin", bufs=1))
        sps = ctx.enter_context(tc.tile_pool(name="spsum", bufs=1,
                                             space="PSUM"))
        pps = ctx.enter_context(tc.tile_pool(name="ppsum", bufs=1,
                                             space="PSUM"))

        ident16 = consts.tile([128, 128], dt.float16)
        make_identity(nc, ident16)
        ident8x2 = consts.tile([128, 2, 128], dt.float8e4)
        make_identity(nc, ident8x2[:, 0, :])
        make_identity(nc, ident8x2[:, 1, :])
        ones = consts.tile([128, 1], dt.float32)
        nc.gpsimd.memset(ones, 1.0)
        warm = consts.tile([128, 512], dt.float16)
        nc.gpsimd.memset(warm, 0.0)
        st_w1 = consts.tile([128, 1], dt.float32)   # sum mask*lse
        st_l2 = consts.tile([128, 1], dt.float32)   # sum mask*picked
        st_m = consts.tile([128, 1], dt.float32)    # sum mask

        # ---------------- resident tensors (layout B) ----------------
        p16 = keep.tile([128, C, G4, W], dt.float16)   # pred, fp16

        # ---------------- early loads ----------------
        t32b = mp.tile([128, G4, W], dt.int32, tag="t32b")
        nc.sync.dma_start(
            out=t32b, in_=target.ap().rearrange("(p r) w -> p r w", p=128))

        # pred chunk DMAs: issue ALL up front on the SWDGE queue so the
        # descriptor stream never bubbles (p16 is resident, no pool deps).
        starts = []
        c0 = 0
        for n in CHUNKS:
            starts.append((c0, n))
            nc.gpsimd.dma_start(
                out=p16[:, c0:c0 + n, :, :],
                in_=pred.ap()[c0:c0 + n].rearrange(
                    "c (p r) w -> p c r w", p=128))
            c0 += n

        # ---------------- mask setup (all layout B, scheduled early) ----
        # padded 8-row tiles: row i = global row 4p + i - 2, cols [2, 514)
        xmax8 = mp.tile([128, 8, PW], dt.float16, tag="xmax8")
        xmin8 = mp.tile([128, 8, PW], dt.float16, tag="xmin8")
        for t, v in ((xmax8, -1.0), (xmin8, 99.0)):
            # w borders (all rows), p0 top rows, p127 bottom rows (the
            # 96:128 memset is 32-aligned; the dn exchange DMA then
            # overwrites partitions 96..126 with real data, leaving 127).
            nc.gpsimd.memset(t[:, :, 0:2], v)
            nc.gpsimd.memset(t[:, :, 2 + W:PW], v)
            nc.gpsimd.memset(t[0:1, 0:2, :], v)
            nc.gpsimd.memset(t[96:128, 6:8, :], v)
        # target -> fp16 directly into both padded tiles (eq reads xmax8)
        tb = xmax8[:, 2:6, 2:2 + W]
        nc.vector.tensor_copy(out=xmax8[:, 2:6, 2:2 + W], in_=t32b)
        nc.vector.tensor_copy(out=xmin8[:, 2:6, 2:2 + W], in_=t32b)

        def st_bdry(x):
            # partition-shift exchanges (sync queue, full padded width):
            # x[p, 0:2] = global rows 4p-2,4p-1 = x[p-1, 4:6]
            nc.sync.dma_start(out=x[1:128, 0:2, :], in_=x[0:127, 4:6, :])
            # x[p, 6:8] = global rows 4p+4,4p+5 = x[p+1, 2:4]
            nc.sync.dma_start(out=x[0:127, 6:8, :], in_=x[1:128, 2:4, :])

        st_bdry(xmax8)
        st_bdry(xmin8)

        # PE warmup into the future P bank (discarded by c==0's start=True)
        s_ps = sps.tile([128, G4, W], dt.float32, tag="s")
        p_ps = pps.tile([128, G4, W], dt.float32, tag="p")
        for _ in range(10):
            nc.tensor.matmul(p_ps[:, 0, :], ident16, warm, start=True,
                             stop=True)

        hx8 = mp.tile([128, 8, W], dt.float16, tag="hx8")
        hn8 = mp.tile([128, 8, W], dt.float16, tag="hn8")
        vx = mp.tile([128, G4, W], dt.float16, tag="vx")
        vn = mp.tile([128, G4, W], dt.float16, tag="vn")
        maskb = keep.tile([128, G4, W], dt.float16)
        junk = mp.tile([128, G4, W], dt.float16, tag="junk")

        def hpool(src, op, dst):
            # horizontal 5-pool over all 8 rows
            m2 = ms.tile([128, 8, PW], dt.float16, tag="m2")
            m4 = ms.tile([128, 8, PW], dt.float16, tag="m4")
            nc.vector.tensor_tensor(
                out=m2[:, :, 0:PW - 1],
                in0=src[:, :, 0:PW - 1], in1=src[:, :, 1:PW], op=op)
            nc.vector.tensor_tensor(
                out=m4[:, :, 0:PW - 3],
                in0=m2[:, :, 0:PW - 3], in1=m2[:, :, 2:PW - 1], op=op)
            nc.vector.tensor_tensor(
                out=dst, in0=m4[:, :, 0:W], in1=src[:, :, 4:4 + W], op=op)

        def vpool(ext, op, dst):
            # vertical 5-pool: out row r needs ext rows r..r+4
            m2 = ms.tile([128, 7, W], dt.float16, tag="v2")
            m4 = ms.tile([128, 5, W], dt.float16, tag="v4")
            nc.vector.tensor_tensor(
                out=m2, in0=ext[:, 0:7, :], in1=ext[:, 1:8, :], op=op)
            nc.vector.tensor_tensor(
                out=m4, in0=m2[:, 0:5, :], in1=m2[:, 2:7, :], op=op)
            nc.vector.tensor_tensor(
                out=dst, in0=m4[:, 0:4, :], in1=ext[:, 4:8, :], op=op)

        def st_neq():
            nc.vector.tensor_tensor(out=maskb, in0=vx, in1=vn,
                                    op=Alu.not_equal)

        def st_msum():
            nc.scalar.activation(out=junk, in_=maskb, func=Act.Copy,
                                 accum_out=st_m)

        stages = [
            lambda: hpool(xmax8, Alu.max, hx8),
            lambda: hpool(xmin8, Alu.min, hn8),
            lambda: vpool(hx8, Alu.max, vx),
            lambda: vpool(hn8, Alu.min, vn),
            st_neq,
            st_msum,
        ]

        # ---------------- class loop, stages interleaved ----------------
        DR = mybir.MatmulPerfMode.DoubleRow
        for k, (c0, nct) in enumerate(starts):
            if k < len(stages):
                stages[k]()
            e8 = epool.tile([128, 4, G4, W], dt.float8e4, tag="e")
            nc.scalar.activation(out=e8[:, 0:nct, :, :],
                                 in_=p16[:, c0:c0 + nct, :, :], func=Act.Exp)
            # S accumulation: DoubleRow over class pairs (fp8, 2 k-tiles)
            for a in range(0, nct - 1, 2):
                for j in range(G4):
                    nc.tensor.matmul(
                        s_ps[:, j, :], ident8x2, e8[:, a:a + 2, j, :],
                        start=(c0 + a == 0), stop=False, perf_mode=DR)
            if nct % 2:  # solo class (the last chunk)
                for j in range(G4):
                    nc.tensor.matmul(
                        s_ps[:, j, :], ident8x2[:, 0, :],
                        e8[:, nct - 1, j, :],
                        start=(c0 + nct - 1 == 0), stop=(c0 + nct == C))
            # picked accumulation: eq (4x) + product (2x) + fp16 matmuls
            for i in range(nct):
                c = c0 + i
                eq_t = opool.tile([128, G4, W], dt.float16, tag="q")
                nc.vector.tensor_scalar(
                    out=eq_t, in0=tb, scalar1=float(c), scalar2=None,
                    op0=Alu.is_equal)
                o_t = opool.tile([128, G4, W], dt.float16, tag="o")
                nc.vector.tensor_tensor(
                    out=o_t, in0=eq_t, in1=p16[:, c, :, :], op=Alu.mult)
                for j in range(G4):
                    nc.tensor.matmul(
                        p_ps[:, j, :], ident16, o_t[:, j, :],
                        start=(c == 0), stop=(c == C - 1))
        for k in range(len(starts), len(stages)):
            stages[k]()

        # ---------------- finals ----------------
        # j2 first on DVE: overlaps the Ln on ACT
        nc.vector.scalar_tensor_tensor(
            out=junk, in0=p_ps, scalar=0.0, in1=maskb,
            op0=Alu.add, op1=Alu.mult, accum_out=st_l2)
        l1 = fin.tile([128, G4, W], dt.float16)
        nc.scalar.activation(out=l1, in_=s_ps, func=Act.Ln)
        nc.vector.scalar_tensor_tensor(
            out=junk, in0=l1, scalar=0.0, in1=maskb,
            op0=Alu.add, op1=Alu.mult, accum_out=st_w1)

        # partition reductions — reuse the S bank (fully consumed by l1)
        red = s_ps[0:1, 0, 0:32]
        nc.tensor.matmul(red[:, 0:1], ones, st_w1, start=True, stop=True)
        nc.tensor.matmul(red[:, 1:2], ones, st_l2, start=True, stop=True)
        nc.tensor.matmul(red[:, 2:3], ones, st_m, start=True, stop=True)
        outsb = consts.tile([1, 32], dt.float32)
        nc.vector.memset(outsb, 0.0)
        nc.vector.tensor_copy(out=outsb[:, 0:3], in_=red[:, 0:3])
        nc.sync.dma_start(out=out.ap(), in_=outsb)

    nc.compile()
    return nc


def get_nc():
    if "nc" not in _CACHE:
        _CACHE["nc"] = _build_nc()
    return _CACHE["nc"]


def _combine(outs):
    """outs: list of per-core [1,32] float32 -> scalar loss."""
    per_sample = []
    for o in outs:
        w1, l2, msum = float(o[0, 0]), float(o[0, 1]), float(o[0, 2])
        wsum = w1 - l2
        if msum > 0:
            per_sample.append(wsum / max(msum, 1.0))
        else:
            per_sample.append(wsum / float(H * W))
    return np.float32(np.mean(per_sample))


def kernel(pred, target):
    from concourse.bass_utils import run_bass_kernel_spmd

    pred = np.ascontiguousarray(pred, dtype=np.float32)
    target = np.ascontiguousarray(target, dtype=np.int32)
    assert pred.shape == (B, C, H, W) and target.shape == (B, H, W)

    nc = get_nc()
    in_maps = [{"pred": pred[b], "target": target[b]} for b in range(B)]
    res = run_bass_kernel_spmd(nc, in_maps, core_ids=list(range(N_CORES)))
    outs = [res.results[b]["out"] for b in range(B)]
    return np.asarray(_combine(outs), dtype=np.float32)
